# revision 56
# baseline (speedup 1.0000x reference)
"""Trainium2 Bass kernel for nn_AttentionRefinementModule (deformable conv + sigmoid).

Data-parallel over batch: 8 samples -> 8 NeuronCores, one full sample per core.

Per-core pipeline (v2):
  P0  xpad: zero-padded input image, bf16, [128p x 2(ch-half) x 4800] in SBUF.
  P1  offset conv on PE over the padded grid -> off [18, 4096] fp32.
  P2  PE-transpose off -> offT [128(pos%128), 32(tile), 18].
  P3  DVE: floor/frac of sample coords; bilinear weights wg (bf16, per
      (pos,tap,corner)); window-relative padded-grid x-pair start indices.
      Out-of-image reads hit zero-padded plane rows/cols, so no validity
      masking beyond the two edge fixes (y0<-1, x0 outside [-1,64]).
  P3b Fold indices into dma_gather's 16-partition-wrapped int16 layout via
      selector matmuls, then log-doubling broadcast to 128 partitions.
  P4  U_k = W_k @ x over the padded grid (36 tiles x 128 pos), PSUM split in
      two halves so the Scalar-engine bf16 cast overlaps the next matmuls;
      per-k planes [9][4608, 256] bf16 streamed to DRAM.
  P5  Per group (2 tiles): 9 dma_gather calls (one per tap, 512 idxs of
      overlapping 1KB x-pair elements, elem_step 256) spread over 4 SWDGE
      queues, windowed in_ap so gathers pipeline behind the UT writes.
      Blend: per tile a 36-step scalar_tensor_tensor chain in bf16
      (acc = gathered*weight + acc), seeded with the broadcast bias row.
  P6  Sigmoid on ScalarE, store [4096, 256]; host transposes to [256, H, W].
"""

import numpy as np

import concourse.bass as bass
import concourse.mybir as mybir
from concourse import bacc
from concourse.tile import TileContext
from concourse.bass_utils import run_bass_kernel_spmd

B, C, H, W = 8, 256, 64, 64
HW = H * W
NK = 9
PW = W + 2           # 66
NPAD = 4800          # xpad free size per channel-half
NT = HW // 128       # 32 interior position tiles
NT2 = 36             # padded-grid tiles (36*128 = 4608)
PLANE = NT2 * 128    # 4608 elems per k-plane
NG = 16              # groups (2 tiles each)
WROWS = 18           # gather window rows per group (in_ap span)
# merged-tap gather call slots: taps per call, one fixed SWDGE queue each
CALLS = [(k, 1) for k in range(NK)]  # (k0, ntaps) -> queue = slot idx % nq
F32 = mybir.dt.float32
BF16 = mybir.dt.bfloat16
I32 = mybir.dt.int32
I16 = mybir.dt.int16
MULT = mybir.AluOpType.mult
ADD = mybir.AluOpType.add

_CACHE = {}


def _r0(g):
    return min(max(4 * g - 5, 0), 50)


def build_nc(nq=4):
    nc = bacc.Bacc(num_swdge_queues=nq)

    x_d = nc.declare_dram_parameter("x", [C, HW], F32, isOutput=False)
    wofflhsT_d = nc.declare_dram_parameter("wofflhsT", [128, 2, NK, 18], BF16, isOutput=False)
    wmov_d = nc.declare_dram_parameter("wmov", [128, 2, NK * 256], BF16, isOutput=False)
    boff_d = nc.declare_dram_parameter("boff", [18, 1], F32, isOutput=False)
    biasrow_d = nc.declare_dram_parameter("biasrow", [128, 256], BF16, isOutput=False)
    ident_d = nc.declare_dram_parameter("ident", [128, 128], F32, isOutput=False)
    basey_d = nc.declare_dram_parameter("basey", [128, NT], F32, isOutput=False)
    basex_d = nc.declare_dram_parameter("basex", [128, 1], F32, isOutput=False)
    kty_d = nc.declare_dram_parameter("kty", [128, NK], F32, isOutput=False)
    ktx_d = nc.declare_dram_parameter("ktx", [128, NK], F32, isOutput=False)
    wloy_d = nc.declare_dram_parameter("wloy", [128, NT], F32, isOutput=False)
    whiy_d = nc.declare_dram_parameter("whiy", [128, NT], F32, isOutput=False)
    qconst_d = nc.declare_dram_parameter("qconst", [128, NT], F32, isOutput=False)
    kofs_d = nc.declare_dram_parameter("kofs", [128, NK], F32, isOutput=False)
    sel_d = nc.declare_dram_parameter("sel", [128, 8, 16], F32, isOutput=False)
    out_d = nc.declare_dram_parameter("out", [HW, C], F32, isOutput=True)

    with TileContext(nc) as tc:
        free_order = []
        free_fns = {}

        def single(name, shape, dt=F32):
            t, fr = tc.tile(shape, dt, name=name)
            free_fns[name] = fr
            free_order.append(name)
            return t

        with (
            tc.tile_pool(name="pstr", bufs=3, space="PSUM") as ps_tr,
            tc.tile_pool(name="psua", bufs=1, space="PSUM") as ps_ua,
            tc.tile_pool(name="psub", bufs=1, space="PSUM") as ps_ub,
            tc.tile_pool(name="dram", bufs=1, space="DRAM") as dpool,
        ):
            ut_d = dpool.tile([NK * PLANE, 256], BF16, name="ut")

            # ---- persistent tiles ----
            # weights duplicated in bf16 pairs so every [128,1] scalar slice
            # is 4B-aligned (DVE 2x perf-mode alignment requirement)
            wg = single("wg", [128, NT * 36, 2], BF16)
            wgf = single("wgf", [128, NT * 36])
            idxw = single("idxw", [128, NG, NK, 32], I16)
            ident = single("ident", [128, 128])
            nc.sync.dma_start(out=ident[:, :], in_=ident_d[:, :])
            woff_sb = single("woff_sb", [128, 2, NK, 18], BF16)
            nc.sync.dma_start(out=woff_sb[:, :, :, :], in_=wofflhsT_d[:, :, :, :])
            wmov_sb = single("wmov_sb", [128, 2, NK * 256], BF16)
            nc.sync.dma_start(out=wmov_sb[:, :, :], in_=wmov_d[:, :, :])
            boff_sb = single("boff_sb", [18, 1])
            nc.sync.dma_start(out=boff_sb[:, :], in_=boff_d[:, :])
            biasrow = single("biasrow", [128, 256], BF16)
            nc.sync.dma_start(out=biasrow[:, :], in_=biasrow_d[:, :])
            zerorow = single("zerorow", [128, 256], BF16)
            nc.vector.memset(zerorow[:, :], 0.0)
            xpad = single("xpad", [128, 2, NPAD], BF16)

            # ---- freeable constants ----
            basey = single("basey", [128, NT])
            nc.sync.dma_start(out=basey[:, :], in_=basey_d[:, :])
            basex = single("basex", [128, 1])
            nc.sync.dma_start(out=basex[:, :], in_=basex_d[:, :])
            kty = single("kty", [128, NK])
            nc.sync.dma_start(out=kty[:, :], in_=kty_d[:, :])
            ktx = single("ktx", [128, NK])
            nc.sync.dma_start(out=ktx[:, :], in_=ktx_d[:, :])
            wloy = single("wloy", [128, NT])
            nc.sync.dma_start(out=wloy[:, :], in_=wloy_d[:, :])
            whiy = single("whiy", [128, NT])
            nc.sync.dma_start(out=whiy[:, :], in_=whiy_d[:, :])
            qconst = single("qconst", [128, NT])
            nc.sync.dma_start(out=qconst[:, :], in_=qconst_d[:, :])
            kofs = single("kofs", [128, NK])
            nc.sync.dma_start(out=kofs[:, :], in_=kofs_d[:, :])
            sel = single("sel", [128, 8, 16])
            nc.sync.dma_start(out=sel[:, :, :], in_=sel_d[:, :, :])

            # ---- P0: padded bf16 input image ----
            nc.gpsimd.memset(xpad[:, :, :], 0.0)
            for g in range(2):
                dst = bass.AP(xpad.tensor, xpad.offset + g * NPAD + PW + 1,
                              [xpad.ap[0], [PW, H], [1, W]])
                nc.gpsimd.dma_start(
                    out=dst,
                    in_=x_d[g * 128:(g + 1) * 128, :].rearrange(
                        "c (h w) -> c h w", w=W))

            psuA = ps_ua.tile([128, 1024], F32, name="psua_t")
            psuB = ps_ub.tile([128, 1280], F32, name="psub_t")

            # ---- P1: offset conv on the padded grid ----
            offp_sb = single("offp_sb", [18, 4608])
            off_sb = single("off_sb", [18, HW])
            for n in range(9):
                ps = (psuA if n % 2 == 0 else psuB)[:18, 0:512]
                first = True
                for g in range(2):
                    for t in range(NK):
                        ty, tx = t // 3, t % 3
                        o0 = ty * PW + tx + n * 512
                        rhs = xpad[:, g, o0:o0 + 512]
                        nc.tensor.matmul(ps, woff_sb[:, g, t, :], rhs,
                                         start=first,
                                         stop=(g == 1 and t == NK - 1))
                        first = False
                nc.scalar.activation(offp_sb[:, n * 512:(n + 1) * 512], ps,
                                     mybir.ActivationFunctionType.Identity,
                                     bias=boff_sb[:, :])
            nc.scalar.activation(
                off_sb.rearrange("j (h w) -> j h w", w=W),
                bass.AP(offp_sb.tensor, offp_sb.offset,
                        [offp_sb.ap[0], [PW, H], [1, W]]),
                mybir.ActivationFunctionType.Copy)

            # ---- P2: transpose off -> offT ----
            offT = single("offT", [128, NT, 18])
            for t in range(NT):
                pst = ps_tr.tile([128, 128], F32, name="ps_tr_t")
                nc.tensor.transpose(pst[:, :18],
                                    off_sb[:, t * 128:(t + 1) * 128],
                                    ident[:18, :18])
                nc.scalar.activation(offT[:, t, :], pst[:, :18],
                                     mybir.ActivationFunctionType.Copy)

            # ---- P3: sample coords, weights, window-relative indices ----
            SH = [128, NT, NK]

            def bcast_tk(ap_pk):  # [128, NK] -> [128, (NT), NK]
                return bass.AP(ap_pk.tensor, ap_pk.offset,
                               [ap_pk.ap[0], [0, NT], ap_pk.ap[1]])

            def bcast_pt(ap_pt):  # [128, NT] -> [128, NT, (NK)]
                return bass.AP(ap_pt.tensor, ap_pt.offset,
                               [ap_pt.ap[0], ap_pt.ap[1], [0, NK]])

            dyx = offT.rearrange("p t (k two) -> p two t k", two=2)
            dy, dx = dyx[:, 0], dyx[:, 1]

            py = single("py", SH)
            px = single("px", SH)
            tA = single("tA", SH)
            nc.vector.tensor_add(tA[:, :, :], dy, bcast_tk(kty[:, :]))
            nc.vector.tensor_add(py[:, :, :], tA[:, :, :], bcast_pt(basey[:, :]))
            nc.vector.tensor_add(tA[:, :, :], dx, bcast_tk(ktx[:, :]))
            nc.vector.tensor_add(px[:, :, :], tA[:, :, :],
                                 bass.AP(basex.tensor, basex.offset,
                                         [basex.ap[0], [0, NT], [0, NK]]))

            def floor_split(p_ap, nm):
                t16 = single(nm + "_t16", SH)
                nc.vector.tensor_scalar_add(t16[:, :, :], p_ap, 16.0)
                ti = single(nm + "_ti", SH, I32)
                nc.vector.tensor_copy(ti[:, :, :], t16[:, :, :])
                tif = single(nm + "_tif", SH)
                nc.vector.tensor_copy(tif[:, :, :], ti[:, :, :])
                fr = single(nm + "_fr", SH)
                nc.vector.tensor_sub(fr[:, :, :], t16[:, :, :], tif[:, :, :])
                ng = single(nm + "_ng", SH)
                nc.vector.tensor_scalar(ng[:, :, :], fr[:, :, :], 0.0, None,
                                        mybir.AluOpType.is_lt)
                w1 = single(nm + "_w1", SH)
                nc.vector.tensor_add(w1[:, :, :], fr[:, :, :], ng[:, :, :])
                t2 = single(nm + "_t2", SH)
                nc.vector.tensor_sub(t2[:, :, :], tif[:, :, :], ng[:, :, :])
                f0 = single(nm + "_f0", SH)
                nc.vector.tensor_scalar_sub(f0[:, :, :], t2[:, :, :], 16.0)
                return f0, w1

            y0, wy1r = floor_split(py[:, :, :], "y")
            x0, wx1r = floor_split(px[:, :, :], "x")

            # edge fixes: corners clamped INTO the image from beyond the pad
            # ring must contribute zero.
            fy = single("fy", SH)
            nc.vector.tensor_scalar(fy[:, :, :], y0[:, :, :], -1.0, None,
                                    mybir.AluOpType.is_ge)
            wy1 = single("wy1", SH)
            nc.vector.tensor_mul(wy1[:, :, :], wy1r[:, :, :], fy[:, :, :])
            fx = single("fx", SH)
            nc.vector.tensor_scalar(fx[:, :, :], x0[:, :, :], -1.0, None,
                                    mybir.AluOpType.is_ge)
            fx2 = single("fx2", SH)
            nc.vector.tensor_scalar(fx2[:, :, :], x0[:, :, :], 64.0, None,
                                    mybir.AluOpType.is_le)
            nc.vector.tensor_mul(fx[:, :, :], fx[:, :, :], fx2[:, :, :])
            wx1 = single("wx1", SH)
            nc.vector.tensor_mul(wx1[:, :, :], wx1r[:, :, :], fx[:, :, :])
            wy0 = single("wy0", SH)
            nc.vector.tensor_scalar(wy0[:, :, :], wy1r[:, :, :], -1.0, 1.0,
                                    MULT, ADD)
            wx0 = single("wx0", SH)
            nc.vector.tensor_scalar(wx0[:, :, :], wx1r[:, :, :], -1.0, 1.0,
                                    MULT, ADD)

            # clamp y0 into the per-group gather window, x0 into [-1, 64]
            y0c = single("y0c", SH)
            nc.vector.tensor_tensor(y0c[:, :, :], y0[:, :, :],
                                    bcast_pt(wloy[:, :]),
                                    op=mybir.AluOpType.max)
            nc.vector.tensor_tensor(y0c[:, :, :], y0c[:, :, :],
                                    bcast_pt(whiy[:, :]),
                                    op=mybir.AluOpType.min)
            x0c = single("x0c", SH)
            nc.vector.tensor_scalar_max(x0c[:, :, :], x0[:, :, :], -1.0)
            nc.vector.tensor_scalar_min(x0c[:, :, :], x0c[:, :, :], 64.0)

            # window-relative x-pair start index (+ per-tap plane offset
            # within its merged gather call)
            qA = single("qA", SH)
            nc.vector.tensor_scalar(qA[:, :, :], y0c[:, :, :], 66.0, None, MULT)
            nc.vector.tensor_add(qA[:, :, :], qA[:, :, :], x0c[:, :, :])
            nc.vector.tensor_add(qA[:, :, :], qA[:, :, :],
                                 bcast_pt(qconst[:, :]))
            nc.vector.tensor_add(qA[:, :, :], qA[:, :, :],
                                 bcast_tk(kofs[:, :]))

            # idxq[p, k, t, a]
            idxq = single("idxq", [128, NK, NT, 2])

            def tk_view(ap3):  # [128, NT, NK] viewed with iteration (k, t)
                return ap3.rearrange("p t k -> p k t")

            nc.vector.tensor_copy(idxq[:, :, :, 0], tk_view(qA[:, :, :]))
            nc.vector.tensor_scalar_add(idxq[:, :, :, 1], tk_view(qA[:, :, :]),
                                        66.0)

            # weights wg[p, t*36 + k*4 + a*2 + b] = wy_a * wx_b (bf16)
            for a, wya in ((0, wy0), (1, wy1)):
                for bb, wxb in ((0, wx0), (1, wx1)):
                    dst = bass.AP(wgf.tensor, wgf.offset + a * 2 + bb,
                                  [wgf.ap[0], [36, NT], [4, NK]])
                    nc.vector.tensor_mul(dst, wya[:, :, :], wxb[:, :, :])
            nc.vector.tensor_copy(
                wg[:, :, :],
                bass.AP(wgf.tensor, wgf.offset,
                        [wgf.ap[0], [1, NT * 36], [0, 2]]))

            # ---- P3b: fold idxq into the wrapped int16 layout ----
            # idxw[b, grp, k, tt*16 + a*8 + g8] = idxq[16*g8+b, k, 2*grp+tt, a]
            for k in range(NK):
                for g8 in range(8):
                    psq = ps_tr.tile([128, 128], F32, name="ps_tr_t")
                    nc.tensor.matmul(psq[:16, :64], sel[:, g8, :],
                                     idxq[:, k, :, :].rearrange(
                                         "p t a -> p (t a)"),
                                     start=True, stop=True)
                    dst = bass.AP(idxw.tensor,
                                  idxw.offset + k * 32 + g8,
                                  [[idxw.ap[0][0], 16], [NK * 32, NG],
                                   [16, 2], [8, 2]])
                    nc.vector.tensor_copy(dst, psq[:16, :64].rearrange(
                        "p (g t a) -> p g t a", t=2, a=2))
            idxw_flat = idxw.rearrange("p g k c -> p (g k c)")
            for step in (16, 32, 64):
                nc.sync.dma_start(out=idxw_flat[step:2 * step, :],
                                  in_=idxw_flat[0:step, :])

            # free P3 temporaries before the steady-state loops
            keep = {"wg", "wgf", "idxw", "ident", "woff_sb", "wmov_sb",
                    "boff_sb", "biasrow", "zerorow", "xpad"}
            for nm in reversed(free_order):
                if nm not in keep:
                    free_fns.pop(nm)()

            # ---- P4 + P5/P6 ----
            with (
                tc.tile_pool(name="utsb", bufs=2) as upool,
                tc.tile_pool(name="gat", bufs=3) as gpool,
                tc.tile_pool(name="accp", bufs=8) as accpool,
                tc.tile_pool(name="tscp", bufs=6) as tpool,
                tc.tile_pool(name="osb", bufs=4) as obpool,
            ):
                # plane positions >= 4352 are all in the bottom pad rows:
                # U is zero there, so write zeros instead of computing them.
                utz = upool.tile([128, NK * 256], BF16, name="utsb_t")
                nc.vector.memset(utz[:, :], 0.0)
                for t in (34, 35):
                    dstz = bass.AP(ut_d.tensor, ut_d.offset + t * 128 * 256,
                                   [[256, 128], [PLANE * 256, NK], [1, 256]])
                    nc.sync.dma_start(
                        out=dstz,
                        in_=utz[:, :].rearrange("p (k c) -> p k c", c=256))
                for t in range(34):
                    for g in range(2):
                        xt = xpad[:, g, t * 128:(t + 1) * 128]
                        nc.tensor.matmul(psuA[:, 0:512], xt,
                                         wmov_sb[:, g, 0:512],
                                         start=(g == 0), stop=(g == 1))
                        nc.tensor.matmul(psuA[:, 512:1024], xt,
                                         wmov_sb[:, g, 512:1024],
                                         start=(g == 0), stop=(g == 1))
                        if g == 1:
                            utsb = upool.tile([128, NK * 256], BF16,
                                              name="utsb_t")
                            nc.scalar.activation(
                                utsb[:, 0:1024], psuA[:, :],
                                mybir.ActivationFunctionType.Copy)
                        for sl in (slice(1024, 1536), slice(1536, 2048),
                                   slice(2048, 2304)):
                            nc.tensor.matmul(
                                psuB[:, sl.start - 1024:sl.stop - 1024], xt,
                                wmov_sb[:, g, sl],
                                start=(g == 0), stop=(g == 1))
                    nc.scalar.activation(utsb[:, 1024:2304], psuB[:, :],
                                         mybir.ActivationFunctionType.Copy)
                    dst = bass.AP(ut_d.tensor, ut_d.offset + t * 128 * 256,
                                  [[256, 128], [PLANE * 256, NK], [1, 256]])
                    nc.sync.dma_start(
                        out=dst,
                        in_=utsb[:, :].rearrange("p (k c) -> p k c", c=256))

                # ---- P5: per-group gathers + split chain blend; P6 store ----
                GPK = 9  # taps >= GPK blend on GpSimd, rest on Vector
                for grp in range(NG):
                    r0 = _r0(grp)
                    gt = gpool.tile([128, NK, 4, 512], BF16, name="gt_t")
                    for q, (k0, nk) in enumerate(CALLS):
                        win = bass.AP(
                            ut_d.tensor,
                            ut_d.offset + (k0 * PLANE + r0 * PW) * 256,
                            [[256, (nk - 1) * PLANE + WROWS * PW], [1, 512]])
                        nc.gpsimd.dma_gather(
                            out_ap=gt[:, k0:k0 + nk, :, :].rearrange(
                                "p k s c -> p (k s) c"),
                            in_ap=win,
                            idxs_ap=idxw[:, grp, k0:k0 + nk, :].rearrange(
                                "p k c -> p (k c)"),
                            num_idxs=nk * 512, num_idxs_reg=nk * 512,
                            elem_size=512, elem_step=256,
                            queue_num=q % nq)
                    accs, acc2s = [], []
                    for tt in range(2):
                        accs.append(accpool.tile([128, 256], BF16,
                                                 name="acc_t"))
                        acc2s.append(accpool.tile([128, 256], BF16,
                                                  name="acc2_t"))
                    # four independent chains (2 tiles x 2 halves) to hide
                    # per-op dependency latency on DVE
                    for j in range(18):
                        for tt in range(2):
                            t = 2 * grp + tt
                            for half, accl in ((0, accs), (1, acc2s)):
                                jj = j + 18 * half
                                k, a, bb = jj // 4, (jj // 2) % 2, jj % 2
                                wsl = wg[:, t * 36 + jj, 0:1]
                                gsl = gt[:, k, tt * 2 + a,
                                         bb * 256:(bb + 1) * 256]
                                if j == 0:
                                    prev = (biasrow[:, :] if half == 0
                                            else zerorow[:, :])
                                else:
                                    prev = accl[tt][:, :]
                                nc.vector.scalar_tensor_tensor(
                                    accl[tt][:, :], gsl, wsl, prev, MULT, ADD)
                    for tt in range(2):
                        t = 2 * grp + tt
                        nc.vector.tensor_add(accs[tt][:, :], accs[tt][:, :],
                                             acc2s[tt][:, :])
                        osb = obpool.tile([128, 256], F32, name="osb_t")
                        nc.scalar.activation(
                            osb[:, :], accs[tt][:, :],
                            mybir.ActivationFunctionType.Sigmoid)
                        nc.sync.dma_start(
                            out=out_d[t * 128:(t + 1) * 128, :],
                            in_=osb[:, :])

            for nm in reversed(free_order):
                if nm in free_fns:
                    free_fns.pop(nm)()

    nc.compile()
    return nc


def prepack(w_off, b_off, w, b):
    import ml_dtypes
    wofflhsT = np.zeros((2, NK, 128, 18), np.float32)
    for g in range(2):
        for t in range(NK):
            ty, tx = t // 3, t % 3
            wofflhsT[g, t] = w_off[:, g * 128:(g + 1) * 128, ty, tx].T
    wmov = np.zeros((2, 128, NK * 256), np.float32)
    for g in range(2):
        for k in range(NK):
            ky, kx = k // 3, k % 3
            wmov[g, :, k * 256:(k + 1) * 256] = w[:, g * 128:(g + 1) * 128, ky, kx].T
    p = np.arange(128)
    basey = (p[:, None] // 64 + 2 * np.arange(NT)[None, :]).astype(np.float32)
    basex = (p[:, None] % 64).astype(np.float32).copy()
    kk = np.arange(NK)
    sel = np.zeros((128, 8, 16), np.float32)
    for a in range(8):
        for bb in range(16):
            sel[16 * a + bb, a, bb] = 1.0
    gidx = np.arange(NT) // 2
    wloy = np.maximum(-1, 4 * gidx - 6).astype(np.float32)
    whiy = np.minimum(64, 4 * gidx + 8).astype(np.float32)
    r0s = np.array([_r0(g) for g in gidx])
    qconst = (67 - 66 * r0s).astype(np.float32)
    k0s = {k: k0 for k0, nk in CALLS for k in range(k0, k0 + nk)}
    kofs = np.array([(k - k0s[k]) * PLANE for k in range(NK)],
                    np.float32)
    return {
        "wofflhsT": np.ascontiguousarray(
            wofflhsT.transpose(2, 0, 1, 3)).astype(ml_dtypes.bfloat16),
        "wmov": np.ascontiguousarray(
            wmov.transpose(1, 0, 2)).astype(ml_dtypes.bfloat16),
        "boff": b_off.reshape(18, 1).astype(np.float32),
        "biasrow": np.broadcast_to(b[None, :], (128, 256)).astype(
            ml_dtypes.bfloat16).copy(),
        "ident": np.eye(128, dtype=np.float32),
        "basey": basey, "basex": basex,
        "kty": np.broadcast_to((kk // 3 - 1).astype(np.float32), (128, NK)).copy(),
        "ktx": np.broadcast_to((kk % 3 - 1).astype(np.float32), (128, NK)).copy(),
        "wloy": np.broadcast_to(wloy[None, :], (128, NT)).copy(),
        "whiy": np.broadcast_to(whiy[None, :], (128, NT)).copy(),
        "qconst": np.broadcast_to(qconst[None, :], (128, NT)).copy(),
        "kofs": np.broadcast_to(kofs[None, :], (128, NK)).copy(),
        "sel": sel,
    }


def make_in_maps(Fstagei, w_off, b_off, w, b):
    shared = prepack(np.asarray(w_off), np.asarray(b_off), np.asarray(w),
                     np.asarray(b))
    in_maps = []
    for i in range(B):
        m = dict(shared)
        m["x"] = np.ascontiguousarray(
            np.asarray(Fstagei[i]).reshape(C, HW).astype(np.float32))
        in_maps.append(m)
    return in_maps


def kernel(Fstagei, w_off, b_off, w, b):
    if "nc" not in _CACHE:
        _CACHE["nc"] = build_nc()
    nc = _CACHE["nc"]
    in_maps = make_in_maps(Fstagei, w_off, b_off, w, b)
    res = run_bass_kernel_spmd(nc, in_maps, core_ids=list(range(B)), trace=False)
    out = np.stack([
        np.ascontiguousarray(
            np.asarray(res.results[i]["out"]).reshape(HW, C).T).reshape(
                C, H, W)
        for i in range(B)])
    return out.astype(np.float32)


# revision 62
# speedup vs baseline: 1.0035x; 1.0035x over previous
"""Trainium2 Bass kernel for nn_AttentionRefinementModule (deformable conv + sigmoid).

Data-parallel over batch: 8 samples -> 8 NeuronCores, one full sample per core.

Per-core pipeline (v2):
  P0  xpad: zero-padded input image, bf16, [128p x 2(ch-half) x 4800] in SBUF.
  P1  offset conv on PE over the padded grid -> off [18, 4096] fp32.
  P2  PE-transpose off -> offT [128(pos%128), 32(tile), 18].
  P3  DVE: floor/frac of sample coords; bilinear weights wg (bf16, per
      (pos,tap,corner)); window-relative padded-grid x-pair start indices.
      Out-of-image reads hit zero-padded plane rows/cols, so no validity
      masking beyond the two edge fixes (y0<-1, x0 outside [-1,64]).
  P3b Fold indices into dma_gather's 16-partition-wrapped int16 layout via
      selector matmuls, then log-doubling broadcast to 128 partitions.
  P4  U_k = W_k @ x over the padded grid (36 tiles x 128 pos), PSUM split in
      two halves so the Scalar-engine bf16 cast overlaps the next matmuls;
      per-k planes [9][4608, 256] bf16 streamed to DRAM.
  P5  Per group (2 tiles): 9 dma_gather calls (one per tap, 512 idxs of
      overlapping 1KB x-pair elements, elem_step 256) spread over 4 SWDGE
      queues, windowed in_ap so gathers pipeline behind the UT writes.
      Blend: per tile a 36-step scalar_tensor_tensor chain in bf16
      (acc = gathered*weight + acc), seeded with the broadcast bias row.
  P6  Sigmoid on ScalarE, store [4096, 256]; host transposes to [256, H, W].
"""

import numpy as np

import concourse.bass as bass
import concourse.mybir as mybir
from concourse import bacc
from concourse.tile import TileContext
from concourse.bass_utils import run_bass_kernel_spmd

B, C, H, W = 8, 256, 64, 64
HW = H * W
NK = 9
PW = W + 2           # 66
NPAD = 4800          # xpad free size per channel-half
NT = HW // 128       # 32 interior position tiles
NT2 = 36             # padded-grid tiles (36*128 = 4608)
PLANE = NT2 * 128    # 4608 elems per k-plane
NG = 16              # groups (2 tiles each)
WROWS = 18           # gather window rows per group (in_ap span)
# merged-tap gather call slots: taps per call, one fixed SWDGE queue each
CALLS = [(k, 1) for k in range(NK)]  # (k0, ntaps) -> queue = slot idx % nq
F32 = mybir.dt.float32
BF16 = mybir.dt.bfloat16
I32 = mybir.dt.int32
I16 = mybir.dt.int16
MULT = mybir.AluOpType.mult
ADD = mybir.AluOpType.add

_CACHE = {}


def _r0(g):
    return min(max(4 * g - 5, 0), 50)


def build_nc(nq=4):
    nc = bacc.Bacc(num_swdge_queues=nq)

    x_d = nc.declare_dram_parameter("x", [C, HW], F32, isOutput=False)
    wofflhsT_d = nc.declare_dram_parameter("wofflhsT", [128, 2, NK, 18], BF16, isOutput=False)
    wmov_d = nc.declare_dram_parameter("wmov", [128, 2, NK * 256], BF16, isOutput=False)
    boff_d = nc.declare_dram_parameter("boff", [18, 1], F32, isOutput=False)
    biasrow_d = nc.declare_dram_parameter("biasrow", [128, 256], BF16, isOutput=False)
    ident_d = nc.declare_dram_parameter("ident", [128, 128], F32, isOutput=False)
    basey_d = nc.declare_dram_parameter("basey", [128, NT], F32, isOutput=False)
    basex_d = nc.declare_dram_parameter("basex", [128, 1], F32, isOutput=False)
    kty_d = nc.declare_dram_parameter("kty", [128, NK], F32, isOutput=False)
    ktx_d = nc.declare_dram_parameter("ktx", [128, NK], F32, isOutput=False)
    wloy_d = nc.declare_dram_parameter("wloy", [128, NT], F32, isOutput=False)
    whiy_d = nc.declare_dram_parameter("whiy", [128, NT], F32, isOutput=False)
    qconst_d = nc.declare_dram_parameter("qconst", [128, NT], F32, isOutput=False)
    kofs_d = nc.declare_dram_parameter("kofs", [128, NK], F32, isOutput=False)
    sel_d = nc.declare_dram_parameter("sel", [128, 8, 16], F32, isOutput=False)
    out_d = nc.declare_dram_parameter("out", [HW, C], F32, isOutput=True)

    with TileContext(nc) as tc:
        free_order = []
        free_fns = {}

        def single(name, shape, dt=F32):
            t, fr = tc.tile(shape, dt, name=name)
            free_fns[name] = fr
            free_order.append(name)
            return t

        with (
            tc.tile_pool(name="pstr", bufs=3, space="PSUM") as ps_tr,
            tc.tile_pool(name="psua", bufs=1, space="PSUM") as ps_ua,
            tc.tile_pool(name="psub", bufs=1, space="PSUM") as ps_ub,
            tc.tile_pool(name="dram", bufs=1, space="DRAM") as dpool,
        ):
            ut_d = dpool.tile([NK * PLANE, 256], BF16, name="ut")

            # ---- persistent tiles ----
            # weights duplicated in bf16 pairs so every [128,1] scalar slice
            # is 4B-aligned (DVE 2x perf-mode alignment requirement)
            wg = single("wg", [128, NT * 36, 2], BF16)
            wgf = single("wgf", [128, NT * 36])
            idxq = single("idxq", [128, NK, NT, 2])
            sel = single("sel", [128, 8, 16])
            nc.sync.dma_start(out=sel[:, :, :], in_=sel_d[:, :, :])
            idxw = single("idxw", [128, NG, NK, 32], I16)
            ident = single("ident", [128, 128])
            nc.sync.dma_start(out=ident[:, :], in_=ident_d[:, :])
            woff_sb = single("woff_sb", [128, 2, NK, 18], BF16)
            nc.sync.dma_start(out=woff_sb[:, :, :, :], in_=wofflhsT_d[:, :, :, :])
            wmov_sb = single("wmov_sb", [128, 2, NK * 256], BF16)
            nc.sync.dma_start(out=wmov_sb[:, :, :], in_=wmov_d[:, :, :])
            boff_sb = single("boff_sb", [18, 1])
            nc.sync.dma_start(out=boff_sb[:, :], in_=boff_d[:, :])
            biasrow = single("biasrow", [128, 256], BF16)
            nc.sync.dma_start(out=biasrow[:, :], in_=biasrow_d[:, :])
            zerorow = single("zerorow", [128, 256], BF16)
            nc.vector.memset(zerorow[:, :], 0.0)
            xpad = single("xpad", [128, 2, NPAD], BF16)

            # ---- freeable constants ----
            basey = single("basey", [128, NT])
            nc.sync.dma_start(out=basey[:, :], in_=basey_d[:, :])
            basex = single("basex", [128, 1])
            nc.sync.dma_start(out=basex[:, :], in_=basex_d[:, :])
            kty = single("kty", [128, NK])
            nc.sync.dma_start(out=kty[:, :], in_=kty_d[:, :])
            ktx = single("ktx", [128, NK])
            nc.sync.dma_start(out=ktx[:, :], in_=ktx_d[:, :])
            wloy = single("wloy", [128, NT])
            nc.sync.dma_start(out=wloy[:, :], in_=wloy_d[:, :])
            whiy = single("whiy", [128, NT])
            nc.sync.dma_start(out=whiy[:, :], in_=whiy_d[:, :])
            qconst = single("qconst", [128, NT])
            nc.sync.dma_start(out=qconst[:, :], in_=qconst_d[:, :])
            kofs = single("kofs", [128, NK])
            nc.sync.dma_start(out=kofs[:, :], in_=kofs_d[:, :])

            # ---- P0: padded bf16 input image ----
            nc.gpsimd.memset(xpad[:, :, :], 0.0)
            for g in range(2):
                dst = bass.AP(xpad.tensor, xpad.offset + g * NPAD + PW + 1,
                              [xpad.ap[0], [PW, H], [1, W]])
                nc.gpsimd.dma_start(
                    out=dst,
                    in_=x_d[g * 128:(g + 1) * 128, :].rearrange(
                        "c (h w) -> c h w", w=W))

            psuA = ps_ua.tile([128, 1024], F32, name="psua_t")
            psuB = ps_ub.tile([128, 1280], F32, name="psub_t")

            # ---- P1: offset conv on the padded grid ----
            offp_sb = single("offp_sb", [18, 4608])
            off_sb = single("off_sb", [18, HW])
            for n in range(9):
                ps = (psuA if n % 2 == 0 else psuB)[:18, 0:512]
                first = True
                for g in range(2):
                    for t in range(NK):
                        ty, tx = t // 3, t % 3
                        o0 = ty * PW + tx + n * 512
                        rhs = xpad[:, g, o0:o0 + 512]
                        nc.tensor.matmul(ps, woff_sb[:, g, t, :], rhs,
                                         start=first,
                                         stop=(g == 1 and t == NK - 1))
                        first = False
                nc.scalar.activation(offp_sb[:, n * 512:(n + 1) * 512], ps,
                                     mybir.ActivationFunctionType.Identity,
                                     bias=boff_sb[:, :])
            nc.scalar.activation(
                off_sb.rearrange("j (h w) -> j h w", w=W),
                bass.AP(offp_sb.tensor, offp_sb.offset,
                        [offp_sb.ap[0], [PW, H], [1, W]]),
                mybir.ActivationFunctionType.Copy)

            # ---- P2: transpose off -> offT ----
            offT = single("offT", [128, NT, 18])
            for t in range(NT):
                pst = ps_tr.tile([128, 128], F32, name="ps_tr_t")
                nc.tensor.transpose(pst[:, :18],
                                    off_sb[:, t * 128:(t + 1) * 128],
                                    ident[:18, :18])
                nc.scalar.activation(offT[:, t, :], pst[:, :18],
                                     mybir.ActivationFunctionType.Copy)

            # ---- P3: sample coords, weights, window-relative indices ----
            SH = [128, NT, NK]

            def bcast_tk(ap_pk):  # [128, NK] -> [128, (NT), NK]
                return bass.AP(ap_pk.tensor, ap_pk.offset,
                               [ap_pk.ap[0], [0, NT], ap_pk.ap[1]])

            def bcast_pt(ap_pt):  # [128, NT] -> [128, NT, (NK)]
                return bass.AP(ap_pt.tensor, ap_pt.offset,
                               [ap_pt.ap[0], ap_pt.ap[1], [0, NK]])

            dyx = offT.rearrange("p t (k two) -> p two t k", two=2)
            dy, dx = dyx[:, 0], dyx[:, 1]

            py = single("py", SH)
            px = single("px", SH)
            tA = single("tA", SH)
            nc.vector.tensor_add(tA[:, :, :], dy, bcast_tk(kty[:, :]))
            nc.vector.tensor_add(py[:, :, :], tA[:, :, :], bcast_pt(basey[:, :]))
            nc.vector.tensor_add(tA[:, :, :], dx, bcast_tk(ktx[:, :]))
            nc.vector.tensor_add(px[:, :, :], tA[:, :, :],
                                 bass.AP(basex.tensor, basex.offset,
                                         [basex.ap[0], [0, NT], [0, NK]]))

            def floor_split(p_ap, nm):
                t16 = single(nm + "_t16", SH)
                nc.vector.tensor_scalar_add(t16[:, :, :], p_ap, 16.0)
                ti = single(nm + "_ti", SH, I32)
                nc.vector.tensor_copy(ti[:, :, :], t16[:, :, :])
                tif = single(nm + "_tif", SH)
                nc.vector.tensor_copy(tif[:, :, :], ti[:, :, :])
                fr = single(nm + "_fr", SH)
                nc.vector.tensor_sub(fr[:, :, :], t16[:, :, :], tif[:, :, :])
                ng = single(nm + "_ng", SH)
                nc.vector.tensor_scalar(ng[:, :, :], fr[:, :, :], 0.0, None,
                                        mybir.AluOpType.is_lt)
                w1 = single(nm + "_w1", SH)
                nc.vector.tensor_add(w1[:, :, :], fr[:, :, :], ng[:, :, :])
                t2 = single(nm + "_t2", SH)
                nc.vector.tensor_sub(t2[:, :, :], tif[:, :, :], ng[:, :, :])
                f0 = single(nm + "_f0", SH)
                nc.vector.tensor_scalar_sub(f0[:, :, :], t2[:, :, :], 16.0)
                return f0, w1

            y0, wy1r = floor_split(py[:, :, :], "y")
            x0, wx1r = floor_split(px[:, :, :], "x")

            # edge fixes: corners clamped INTO the image from beyond the pad
            # ring must contribute zero.
            fy = single("fy", SH)
            nc.vector.tensor_scalar(fy[:, :, :], y0[:, :, :], -1.0, None,
                                    mybir.AluOpType.is_ge)
            wy1 = single("wy1", SH)
            nc.vector.tensor_mul(wy1[:, :, :], wy1r[:, :, :], fy[:, :, :])
            fx = single("fx", SH)
            nc.vector.tensor_scalar(fx[:, :, :], x0[:, :, :], -1.0, None,
                                    mybir.AluOpType.is_ge)
            fx2 = single("fx2", SH)
            nc.vector.tensor_scalar(fx2[:, :, :], x0[:, :, :], 64.0, None,
                                    mybir.AluOpType.is_le)
            nc.vector.tensor_mul(fx[:, :, :], fx[:, :, :], fx2[:, :, :])
            wx1 = single("wx1", SH)
            nc.vector.tensor_mul(wx1[:, :, :], wx1r[:, :, :], fx[:, :, :])
            wy0 = single("wy0", SH)
            nc.vector.tensor_scalar(wy0[:, :, :], wy1r[:, :, :], -1.0, 1.0,
                                    MULT, ADD)
            wx0 = single("wx0", SH)
            nc.vector.tensor_scalar(wx0[:, :, :], wx1r[:, :, :], -1.0, 1.0,
                                    MULT, ADD)

            # clamp y0 into the per-group gather window, x0 into [-1, 64]
            y0c = single("y0c", SH)
            nc.vector.tensor_tensor(y0c[:, :, :], y0[:, :, :],
                                    bcast_pt(wloy[:, :]),
                                    op=mybir.AluOpType.max)
            nc.vector.tensor_tensor(y0c[:, :, :], y0c[:, :, :],
                                    bcast_pt(whiy[:, :]),
                                    op=mybir.AluOpType.min)
            x0c = single("x0c", SH)
            nc.vector.tensor_scalar_max(x0c[:, :, :], x0[:, :, :], -1.0)
            nc.vector.tensor_scalar_min(x0c[:, :, :], x0c[:, :, :], 64.0)

            # window-relative x-pair start index (+ per-tap plane offset
            # within its merged gather call)
            qA = single("qA", SH)
            nc.vector.tensor_scalar(qA[:, :, :], y0c[:, :, :], 66.0, None, MULT)
            nc.vector.tensor_add(qA[:, :, :], qA[:, :, :], x0c[:, :, :])
            nc.vector.tensor_add(qA[:, :, :], qA[:, :, :],
                                 bcast_pt(qconst[:, :]))
            nc.vector.tensor_add(qA[:, :, :], qA[:, :, :],
                                 bcast_tk(kofs[:, :]))

            # idxq[p, k, t, a]
            def tk_view(ap3):  # [128, NT, NK] viewed with iteration (k, t)
                return ap3.rearrange("p t k -> p k t")

            nc.vector.tensor_copy(idxq[:, :, :, 0], tk_view(qA[:, :, :]))
            nc.vector.tensor_scalar_add(idxq[:, :, :, 1], tk_view(qA[:, :, :]),
                                        66.0)

            # weights wg[p, t*36 + k*4 + a*2 + b] = wy_a * wx_b (bf16)
            for a, wya in ((0, wy0), (1, wy1)):
                for bb, wxb in ((0, wx0), (1, wx1)):
                    dst = bass.AP(wgf.tensor, wgf.offset + a * 2 + bb,
                                  [wgf.ap[0], [36, NT], [4, NK]])
                    nc.vector.tensor_mul(dst, wya[:, :, :], wxb[:, :, :])
            nc.vector.tensor_copy(
                wg[:, :, :],
                bass.AP(wgf.tensor, wgf.offset,
                        [wgf.ap[0], [1, NT * 36], [0, 2]]))

            # free P3 temporaries before the steady-state loops (idxq/sel
            # survive: P3b is emitted between early P4 tiles below)
            keep = {"wg", "wgf", "idxw", "idxq", "sel", "ident", "woff_sb",
                    "wmov_sb", "boff_sb", "biasrow", "zerorow", "xpad"}
            for nm in reversed(free_order):
                if nm not in keep:
                    free_fns.pop(nm)()

            # ---- P4 + P5/P6 ----
            with (
                tc.tile_pool(name="utsb", bufs=2) as upool,
                tc.tile_pool(name="gat", bufs=3) as gpool,
                tc.tile_pool(name="accp", bufs=8) as accpool,
                tc.tile_pool(name="tscp", bufs=6) as tpool,
                tc.tile_pool(name="osb", bufs=4) as obpool,
            ):
                # plane positions >= 4352 are all in the bottom pad rows:
                # U is zero there, so write zeros instead of computing them.
                utz = upool.tile([128, NK * 256], BF16, name="utsb_t")
                nc.vector.memset(utz[:, :], 0.0)
                for t in (34, 35):
                    dstz = bass.AP(ut_d.tensor, ut_d.offset + t * 128 * 256,
                                   [[256, 128], [PLANE * 256, NK], [1, 256]])
                    nc.sync.dma_start(
                        out=dstz,
                        in_=utz[:, :].rearrange("p (k c) -> p k c", c=256))
                def p4_tile(t):
                    utsb = None
                    for g in range(2):
                        xt = xpad[:, g, t * 128:(t + 1) * 128]
                        nc.tensor.matmul(psuA[:, 0:512], xt,
                                         wmov_sb[:, g, 0:512],
                                         start=(g == 0), stop=(g == 1))
                        nc.tensor.matmul(psuA[:, 512:1024], xt,
                                         wmov_sb[:, g, 512:1024],
                                         start=(g == 0), stop=(g == 1))
                        if g == 1:
                            utsb = upool.tile([128, NK * 256], BF16,
                                              name="utsb_t")
                            nc.scalar.activation(
                                utsb[:, 0:1024], psuA[:, :],
                                mybir.ActivationFunctionType.Copy)
                        for sl in (slice(1024, 1536), slice(1536, 2048),
                                   slice(2048, 2304)):
                            nc.tensor.matmul(
                                psuB[:, sl.start - 1024:sl.stop - 1024], xt,
                                wmov_sb[:, g, sl],
                                start=(g == 0), stop=(g == 1))
                    nc.scalar.activation(utsb[:, 1024:2304], psuB[:, :],
                                         mybir.ActivationFunctionType.Copy)
                    dst = bass.AP(ut_d.tensor, ut_d.offset + t * 128 * 256,
                                  [[256, 128], [PLANE * 256, NK], [1, 256]])
                    nc.sync.dma_start(
                        out=dst,
                        in_=utsb[:, :].rearrange("p (k c) -> p k c", c=256))

                for t in range(12):
                    p4_tile(t)

                # ---- P3b: fold idxq into dma_gather's wrapped int16 layout
                # (emitted after the first P4 tiles so the PE keeps busy
                # while P3 runs on the vector engine)
                # idxw[b, grp, k, tt*16+a*8+g8] = idxq[16*g8+b, k, 2grp+tt, a]
                for k in range(NK):
                    for g8 in range(8):
                        psq = ps_tr.tile([128, 128], F32, name="ps_tr_t")
                        nc.tensor.matmul(psq[:16, :64], sel[:, g8, :],
                                         idxq[:, k, :, :].rearrange(
                                             "p t a -> p (t a)"),
                                         start=True, stop=True)
                        dst = bass.AP(idxw.tensor,
                                      idxw.offset + k * 32 + g8,
                                      [[idxw.ap[0][0], 16], [NK * 32, NG],
                                       [16, 2], [8, 2]])
                        nc.vector.tensor_copy(dst, psq[:16, :64].rearrange(
                            "p (g t a) -> p g t a", t=2, a=2))
                idxw_flat = idxw.rearrange("p g k c -> p (g k c)")
                for step in (16, 32, 64):
                    nc.sync.dma_start(out=idxw_flat[step:2 * step, :],
                                      in_=idxw_flat[0:step, :])

                for t in range(12, 34):
                    p4_tile(t)

                # ---- P5: per-group gathers + split chain blend; P6 store ----
                GPK = 9  # taps >= GPK blend on GpSimd, rest on Vector
                for grp in range(NG):
                    r0 = _r0(grp)
                    gt = gpool.tile([128, NK, 4, 512], BF16, name="gt_t")
                    for q, (k0, nk) in enumerate(CALLS):
                        win = bass.AP(
                            ut_d.tensor,
                            ut_d.offset + (k0 * PLANE + r0 * PW) * 256,
                            [[256, (nk - 1) * PLANE + WROWS * PW], [1, 512]])
                        nc.gpsimd.dma_gather(
                            out_ap=gt[:, k0:k0 + nk, :, :].rearrange(
                                "p k s c -> p (k s) c"),
                            in_ap=win,
                            idxs_ap=idxw[:, grp, k0:k0 + nk, :].rearrange(
                                "p k c -> p (k c)"),
                            num_idxs=nk * 512, num_idxs_reg=nk * 512,
                            elem_size=512, elem_step=256,
                            queue_num=q % nq)
                    accs = []
                    for tt in range(2):
                        accs.append(accpool.tile([128, 256], BF16,
                                                 name="acc_t"))
                    for j in range(36):
                        k, a, bb = j // 4, (j // 2) % 2, j % 2
                        for tt in range(2):
                            t = 2 * grp + tt
                            wsl = wg[:, t * 36 + j, 0:1]
                            gsl = gt[:, k, tt * 2 + a,
                                     bb * 256:(bb + 1) * 256]
                            prev = biasrow[:, :] if j == 0 else accs[tt][:, :]
                            nc.vector.scalar_tensor_tensor(
                                accs[tt][:, :], gsl, wsl, prev, MULT, ADD)
                    for tt in range(2):
                        t = 2 * grp + tt
                        osb = obpool.tile([128, 256], F32, name="osb_t")
                        nc.scalar.activation(
                            osb[:, :], accs[tt][:, :],
                            mybir.ActivationFunctionType.Sigmoid)
                        nc.sync.dma_start(
                            out=out_d[t * 128:(t + 1) * 128, :],
                            in_=osb[:, :])

            for nm in reversed(free_order):
                if nm in free_fns:
                    free_fns.pop(nm)()

    nc.compile()
    return nc


def prepack(w_off, b_off, w, b):
    import ml_dtypes
    wofflhsT = np.zeros((2, NK, 128, 18), np.float32)
    for g in range(2):
        for t in range(NK):
            ty, tx = t // 3, t % 3
            wofflhsT[g, t] = w_off[:, g * 128:(g + 1) * 128, ty, tx].T
    wmov = np.zeros((2, 128, NK * 256), np.float32)
    for g in range(2):
        for k in range(NK):
            ky, kx = k // 3, k % 3
            wmov[g, :, k * 256:(k + 1) * 256] = w[:, g * 128:(g + 1) * 128, ky, kx].T
    p = np.arange(128)
    basey = (p[:, None] // 64 + 2 * np.arange(NT)[None, :]).astype(np.float32)
    basex = (p[:, None] % 64).astype(np.float32).copy()
    kk = np.arange(NK)
    sel = np.zeros((128, 8, 16), np.float32)
    for a in range(8):
        for bb in range(16):
            sel[16 * a + bb, a, bb] = 1.0
    gidx = np.arange(NT) // 2
    wloy = np.maximum(-1, 4 * gidx - 6).astype(np.float32)
    whiy = np.minimum(64, 4 * gidx + 8).astype(np.float32)
    r0s = np.array([_r0(g) for g in gidx])
    qconst = (67 - 66 * r0s).astype(np.float32)
    k0s = {k: k0 for k0, nk in CALLS for k in range(k0, k0 + nk)}
    kofs = np.array([(k - k0s[k]) * PLANE for k in range(NK)],
                    np.float32)
    return {
        "wofflhsT": np.ascontiguousarray(
            wofflhsT.transpose(2, 0, 1, 3)).astype(ml_dtypes.bfloat16),
        "wmov": np.ascontiguousarray(
            wmov.transpose(1, 0, 2)).astype(ml_dtypes.bfloat16),
        "boff": b_off.reshape(18, 1).astype(np.float32),
        "biasrow": np.broadcast_to(b[None, :], (128, 256)).astype(
            ml_dtypes.bfloat16).copy(),
        "ident": np.eye(128, dtype=np.float32),
        "basey": basey, "basex": basex,
        "kty": np.broadcast_to((kk // 3 - 1).astype(np.float32), (128, NK)).copy(),
        "ktx": np.broadcast_to((kk % 3 - 1).astype(np.float32), (128, NK)).copy(),
        "wloy": np.broadcast_to(wloy[None, :], (128, NT)).copy(),
        "whiy": np.broadcast_to(whiy[None, :], (128, NT)).copy(),
        "qconst": np.broadcast_to(qconst[None, :], (128, NT)).copy(),
        "kofs": np.broadcast_to(kofs[None, :], (128, NK)).copy(),
        "sel": sel,
    }


def make_in_maps(Fstagei, w_off, b_off, w, b):
    shared = prepack(np.asarray(w_off), np.asarray(b_off), np.asarray(w),
                     np.asarray(b))
    in_maps = []
    for i in range(B):
        m = dict(shared)
        m["x"] = np.ascontiguousarray(
            np.asarray(Fstagei[i]).reshape(C, HW).astype(np.float32))
        in_maps.append(m)
    return in_maps


def kernel(Fstagei, w_off, b_off, w, b):
    if "nc" not in _CACHE:
        _CACHE["nc"] = build_nc()
    nc = _CACHE["nc"]
    in_maps = make_in_maps(Fstagei, w_off, b_off, w, b)
    res = run_bass_kernel_spmd(nc, in_maps, core_ids=list(range(B)), trace=False)
    out = np.stack([
        np.ascontiguousarray(
            np.asarray(res.results[i]["out"]).reshape(HW, C).T).reshape(
                C, H, W)
        for i in range(B)])
    return out.astype(np.float32)
